# revision 1
# baseline (speedup 1.0000x reference)
"""BitAttention (BitNet-style ternary-quantized attention) on 8 Trainium2
NeuronCores.

Sharding: tensor-parallel across heads. 16 heads / 8 cores = 2 heads per
core. Each core computes q/k/v projections for its 2 heads (output-dim
shard), causal attention for those heads, and a partial out-projection
(input-dim shard of wo). Host sums the 8 partial outputs (the all-reduce
of the hint, done at unshard time).

Weight quantization sign(w) * mean(|w|) is separable: the +-1 sign
matrices are exact in bf16 and become matmul operands; the four scalar
scales are folded into the softmax exp scale and the output scale, both
applied on-device in fp32 via tiny input tensors.

Layouts (per core):
  xT   [D, B*T]  x transposed (host), bf16 - moving operand of q/k proj,
                 stationary of v proj.
  q^T,k^T kept [head_dim, tok] in SBUF; V kept [tok, head_dim];
  scores computed transposed S^T = [k-tok, q-tok] so that
  P^T = exp(S^T) feeds the y^T matmul directly (no on-chip transposes
  anywhere). Softmax denominator accumulated on DVE, reduced across
  partitions on GpSimd, broadcast back via a rank-1 matmul.
"""

import numpy as np
import ml_dtypes

B, T, D, H = 2, 2048, 2048, 16
HD = 128  # head dim
NCORES = 8
HPC = H // NCORES  # heads per core = 2
HDC = HPC * HD  # per-core projection width = 256
BT = B * T  # 4096

QT = 512  # q-tile (free dim of S^T / y^T matmuls)
KB = 128  # k-block (partition dim of S^T)

_cache = {}


def _build_nc():
    import concourse.tile as tile
    from concourse import bacc, mybir

    f32 = mybir.dt.float32
    bf16 = mybir.dt.bfloat16

    nc = bacc.Bacc("TRN2", target_bir_lowering=False, debug=False,
                   num_devices=NCORES)

    xT = nc.dram_tensor("xT", [D, BT], bf16, kind="ExternalInput").ap()
    wqT = nc.dram_tensor("wqT", [D, HDC], bf16, kind="ExternalInput").ap()
    wkT = nc.dram_tensor("wkT", [D, HDC], bf16, kind="ExternalInput").ap()
    wvT = nc.dram_tensor("wvT", [D, HDC], bf16, kind="ExternalInput").ap()
    woT = nc.dram_tensor("woT", [HDC, D], bf16, kind="ExternalInput").ap()
    # scal_qk: [128,1] filled with s_q*s_k/sqrt(HD) (exp scale)
    # scal_vo: [1,128] filled with s_v*s_o (folded into 1/d broadcast)
    scal_qk = nc.dram_tensor("scal_qk", [128, 1], f32, kind="ExternalInput").ap()
    scal_vo = nc.dram_tensor("scal_vo", [1, 128], f32, kind="ExternalInput").ap()
    out = nc.dram_tensor("out", [BT, D], bf16, kind="ExternalOutput").ap()

    with tile.TileContext(nc) as tc:
        with (
            tc.tile_pool(name="singles", bufs=1) as singles,
            tc.tile_pool(name="xstream", bufs=2) as xstream,
            tc.tile_pool(name="work", bufs=3) as work,
            tc.tile_pool(name="dwork", bufs=2) as dwork,
            tc.tile_pool(name="outsb", bufs=4) as outsb,
        ):
            # ---- persistent SBUF tensors -------------------------------
            wq_sb = singles.tile([128, D // 128, HDC], bf16, tag="wq")
            wk_sb = singles.tile([128, D // 128, HDC], bf16, tag="wk")
            wv_sb = singles.tile([128, D // 128, HDC], bf16, tag="wv")
            wo_sb = singles.tile([128, HPC, D], bf16, tag="wo")
            sqk_sb = singles.tile([128, 1], f32, tag="sqk")
            svo_sb = singles.tile([1, 128], f32, tag="svo")
            qT_sb = singles.tile([128, HPC, BT], bf16, tag="qT")
            kT_sb = singles.tile([128, HPC, BT], bf16, tag="kT")
            v_sb = singles.tile([128, BT // 128, HDC], bf16, tag="v")
            yT_sb = singles.tile([128, HPC, BT], bf16, tag="yT")

            nc.sync.dma_start(out=wq_sb, in_=wqT.rearrange("(c p) m -> p c m", p=128))
            nc.sync.dma_start(out=wk_sb, in_=wkT.rearrange("(c p) m -> p c m", p=128))
            nc.sync.dma_start(out=wv_sb, in_=wvT.rearrange("(c p) m -> p c m", p=128))
            nc.sync.dma_start(out=wo_sb, in_=woT.rearrange("(c p) m -> p c m", p=128))
            nc.sync.dma_start(out=sqk_sb, in_=scal_qk)
            nc.sync.dma_start(out=svo_sb, in_=scal_vo)

            # ---- stage A: projections ---------------------------------
            NKK = D // 128  # 16 contraction chunks
            xTr = xT.rearrange("(c p) n -> p c n", p=128)
            with tc.tile_pool(name="psA", bufs=2, space="PSUM") as psA:
                for j in range(BT // QT):  # 8 column blocks of 512 tokens
                    xt = xstream.tile([128, NKK, QT], bf16, tag="xt")
                    nc.sync.dma_start(out=xt, in_=xTr[:, :, j * QT:(j + 1) * QT])
                    # q^T, k^T : [head_dim part, tok free]
                    for (wsb, dst) in ((wq_sb, qT_sb), (wk_sb, kT_sb)):
                        for h2 in range(HPC):
                            ps = psA.tile([128, QT], f32, tag="qk")
                            for kk in range(NKK):
                                nc.tensor.matmul(
                                    ps,
                                    lhsT=wsb[:, kk, h2 * HD:(h2 + 1) * HD],
                                    rhs=xt[:, kk, :],
                                    start=(kk == 0), stop=(kk == NKK - 1),
                                )
                            nc.any.tensor_copy(
                                dst[:, h2, j * QT:(j + 1) * QT], ps)
                    # v : [tok part, head_dim free]
                    for mm in range(QT // 128):  # 4 token chunks of 128
                        ps = psA.tile([128, HDC], f32, tag="v")
                        for kk in range(NKK):
                            nc.tensor.matmul(
                                ps,
                                lhsT=xt[:, kk, mm * 128:(mm + 1) * 128],
                                rhs=wv_sb[:, kk, :],
                                start=(kk == 0), stop=(kk == NKK - 1),
                            )
                        nc.any.tensor_copy(v_sb[:, j * 4 + mm, :], ps)

            # ---- stage B: causal attention ----------------------------
            from concourse.mybir import ActivationFunctionType as AF
            from concourse.mybir import AluOpType, AxisListType

            with (
                tc.tile_pool(name="psS", bufs=2, space="PSUM") as psS,
                tc.tile_pool(name="psY", bufs=2, space="PSUM") as psY,
                tc.tile_pool(name="psR", bufs=2, space="PSUM") as psR,
            ):
                for b in range(B):
                    for hh in range(HPC):
                        t0 = b * T
                        for iq in range(T // QT):  # 4 q-tiles of 512
                            nkb = (iq + 1) * (QT // KB)
                            y_ps = psY.tile([128, QT], f32, tag="y")
                            d_acc = dwork.tile([128, QT], f32, tag="dacc")
                            for kb in range(nkb):
                                s_ps = psS.tile([128, QT], f32, tag="s")
                                nc.tensor.matmul(
                                    s_ps,
                                    lhsT=kT_sb[:, hh, t0 + kb * KB:t0 + (kb + 1) * KB],
                                    rhs=qT_sb[:, hh, t0 + iq * QT:t0 + (iq + 1) * QT],
                                    start=True, stop=True,
                                )
                                pt = work.tile([128, QT], bf16, tag="pt")
                                nc.scalar.activation(pt, s_ps, AF.Exp, scale=sqk_sb)
                                if kb >= iq * (QT // KB):
                                    # diagonal block: zero where k_glob > q_glob
                                    nc.gpsimd.affine_select(
                                        out=pt, in_=pt,
                                        pattern=[[1, QT]],
                                        channel_multiplier=-1,
                                        base=iq * QT - kb * KB,
                                        compare_op=AluOpType.is_ge,
                                        fill=0.0,
                                    )
                                if kb == 0:
                                    nc.vector.tensor_copy(d_acc, pt)
                                else:
                                    nc.vector.tensor_add(d_acc, d_acc, pt)
                                nc.tensor.matmul(
                                    y_ps,
                                    lhsT=v_sb[:, b * (T // 128) + kb,
                                              hh * HD:(hh + 1) * HD],
                                    rhs=pt,
                                    start=(kb == 0), stop=(kb == nkb - 1),
                                )
                            # softmax denominator and normalization
                            d_row = dwork.tile([1, QT], f32, tag="drow")
                            nc.gpsimd.reduce_sum(out=d_row, in_=d_acc,
                                                 axis=AxisListType.C)
                            dr = dwork.tile([1, QT], f32, tag="dr")
                            nc.vector.reciprocal(dr, d_row)
                            r_ps = psR.tile([128, QT], f32, tag="r")
                            # rank-1 broadcast: R[p, q] = s_v*s_o / d[q]
                            nc.tensor.matmul(r_ps, lhsT=svo_sb, rhs=dr,
                                             start=True, stop=True)
                            r_sb = dwork.tile([128, QT], f32, tag="rsb")
                            nc.scalar.copy(r_sb, r_ps)
                            nc.vector.tensor_mul(
                                yT_sb[:, hh, t0 + iq * QT:t0 + (iq + 1) * QT],
                                y_ps, r_sb)

            # ---- stage C: out-projection (partial) --------------------
            with tc.tile_pool(name="psC", bufs=4, space="PSUM") as psC:
                for m in range(BT // 128):  # 32 token chunks
                    for n in range(D // QT):  # 4 output column tiles
                        ps = psC.tile([128, QT], f32, tag="o")
                        for kk in range(HPC):
                            nc.tensor.matmul(
                                ps,
                                lhsT=yT_sb[:, kk, m * 128:(m + 1) * 128],
                                rhs=wo_sb[:, kk, n * QT:(n + 1) * QT],
                                start=(kk == 0), stop=(kk == HPC - 1),
                            )
                        o_sb = outsb.tile([128, QT], bf16, tag="osb")
                        nc.any.tensor_copy(o_sb, ps)
                        nc.sync.dma_start(
                            out=out[m * 128:(m + 1) * 128, n * QT:(n + 1) * QT],
                            in_=o_sb)

    nc.compile()
    return nc


def kernel(x, wq, wk, wv, wo):
    import concourse.bass_utils as bass_utils

    x = np.asarray(x, dtype=np.float32)
    bf16 = ml_dtypes.bfloat16

    if "nc" not in _cache:
        _cache["nc"] = _build_nc()
    nc = _cache["nc"]

    scales = {}
    signs = {}
    for name, w in (("q", wq), ("k", wk), ("v", wv), ("o", wo)):
        w = np.asarray(w, dtype=np.float32)
        scales[name] = max(np.mean(np.abs(w)), 1e-5)
        signs[name] = np.sign(w)

    s_qk = np.float32(scales["q"] * scales["k"] / np.sqrt(HD))
    s_vo = np.float32(scales["v"] * scales["o"])

    xT = np.ascontiguousarray(x.reshape(BT, D).T).astype(bf16)
    scal_qk = np.full((128, 1), s_qk, dtype=np.float32)
    scal_vo = np.full((1, 128), s_vo, dtype=np.float32)

    in_maps = []
    for c in range(NCORES):
        sl = slice(c * HDC, (c + 1) * HDC)
        in_maps.append({
            "xT": xT,
            "wqT": np.ascontiguousarray(signs["q"][sl, :].T).astype(bf16),
            "wkT": np.ascontiguousarray(signs["k"][sl, :].T).astype(bf16),
            "wvT": np.ascontiguousarray(signs["v"][sl, :].T).astype(bf16),
            "woT": np.ascontiguousarray(signs["o"][:, sl].T).astype(bf16),
            "scal_qk": scal_qk,
            "scal_vo": scal_vo,
        })

    res = bass_utils.run_bass_kernel_spmd(nc, in_maps,
                                          core_ids=list(range(NCORES)),
                                          **_cache.get("run_kwargs", {}))
    _cache["last_result"] = res

    acc = np.zeros((BT, D), dtype=np.float32)
    for r in res.results:
        acc += np.asarray(r["out"], dtype=np.float32)
    return acc.reshape(B, T, D)


# revision 3
# speedup vs baseline: 2.7919x; 2.7919x over previous
"""BitAttention (BitNet-style ternary-quantized attention) on 8 Trainium2
NeuronCores.

Sharding: tensor-parallel across heads. 16 heads / 8 cores = 2 heads per
core. Each core computes q/k/v projections for its 2 heads (output-dim
shard), causal attention for those heads, and a partial out-projection
(input-dim shard of wo). Host sums the 8 partial outputs (the all-reduce
of the hint, done at unshard time).

Weight quantization sign(w) * mean(|w|) is separable: the +-1 sign
matrices are exact in bf16 and become matmul operands; the four scalar
scales are folded into the softmax exp scale and the output scale, both
applied on-device in fp32 via tiny input tensors.

Layouts (per core):
  xT   [D, B*T]  x transposed (host), bf16 - moving operand of q/k proj,
                 stationary of v proj.
  q^T,k^T kept [head_dim, tok] in SBUF; V kept [tok, head_dim];
  scores computed transposed S^T = [k-tok, q-tok] so that
  P^T = exp(S^T) feeds the y^T matmul directly (no on-chip transposes
  anywhere). Softmax denominator accumulated on DVE, reduced across
  partitions on GpSimd, broadcast back via a rank-1 matmul.
"""

import numpy as np
import ml_dtypes

B, T, D, H = 2, 2048, 2048, 16
HD = 128  # head dim
NCORES = 8
HPC = H // NCORES  # heads per core = 2
HDC = HPC * HD  # per-core projection width = 256
BT = B * T  # 4096

QT = 512  # q-tile (free dim of S^T / y^T matmuls)
KB = 128  # k-block (partition dim of S^T)

_cache = {}


def _build_nc():
    import concourse.tile as tile
    from concourse import bacc, mybir

    f32 = mybir.dt.float32
    bf16 = mybir.dt.bfloat16

    nc = bacc.Bacc("TRN2", target_bir_lowering=False, debug=False,
                   num_devices=NCORES)

    xT = nc.dram_tensor("xT", [D, BT], bf16, kind="ExternalInput").ap()
    wqT = nc.dram_tensor("wqT", [D, HDC], bf16, kind="ExternalInput").ap()
    wkT = nc.dram_tensor("wkT", [D, HDC], bf16, kind="ExternalInput").ap()
    wvT = nc.dram_tensor("wvT", [D, HDC], bf16, kind="ExternalInput").ap()
    woT = nc.dram_tensor("woT", [HDC, D], bf16, kind="ExternalInput").ap()
    # scal_qk: [128,1] filled with s_q*s_k/sqrt(HD) (exp scale)
    # scal_vo: [1,128] filled with s_v*s_o (folded into 1/d broadcast)
    scal_qk = nc.dram_tensor("scal_qk", [128, 1], f32, kind="ExternalInput").ap()
    scal_vo = nc.dram_tensor("scal_vo", [1, 128], f32, kind="ExternalInput").ap()
    out = nc.dram_tensor("out", [BT, D], bf16, kind="ExternalOutput").ap()

    with tile.TileContext(nc) as tc:
        with (
            tc.tile_pool(name="singles", bufs=1) as singles,
            tc.tile_pool(name="xstream", bufs=2) as xstream,
            tc.tile_pool(name="work", bufs=3) as work,
            tc.tile_pool(name="dwork", bufs=2) as dwork,
            tc.tile_pool(name="outsb", bufs=4) as outsb,
        ):
            # ---- persistent SBUF tensors -------------------------------
            wq_sb = singles.tile([128, D // 128, HDC], bf16, tag="wq")
            wk_sb = singles.tile([128, D // 128, HDC], bf16, tag="wk")
            wv_sb = singles.tile([128, D // 128, HDC], bf16, tag="wv")
            wo_sb = singles.tile([128, HPC, D], bf16, tag="wo")
            sqk_sb = singles.tile([128, 1], f32, tag="sqk")
            svo_sb = singles.tile([1, 128], f32, tag="svo")
            ones_col = singles.tile([128, 1], bf16, tag="ones")
            nc.vector.memset(ones_col, 1.0)
            qT_sb = singles.tile([128, HPC, BT], bf16, tag="qT")
            kT_sb = singles.tile([128, HPC, BT], bf16, tag="kT")
            v_sb = singles.tile([128, BT // 128, HDC], bf16, tag="v")
            yT_sb = singles.tile([128, HPC, BT], bf16, tag="yT")

            nc.sync.dma_start(out=wq_sb, in_=wqT.rearrange("(c p) m -> p c m", p=128))
            nc.sync.dma_start(out=wk_sb, in_=wkT.rearrange("(c p) m -> p c m", p=128))
            nc.sync.dma_start(out=wv_sb, in_=wvT.rearrange("(c p) m -> p c m", p=128))
            nc.sync.dma_start(out=wo_sb, in_=woT.rearrange("(c p) m -> p c m", p=128))
            nc.sync.dma_start(out=sqk_sb, in_=scal_qk)
            nc.sync.dma_start(out=svo_sb, in_=scal_vo)

            # ---- stage A: projections ---------------------------------
            NKK = D // 128  # 16 contraction chunks
            xTr = xT.rearrange("(c p) n -> p c n", p=128)
            with tc.tile_pool(name="psA", bufs=2, space="PSUM") as psA:
                for j in range(BT // QT):  # 8 column blocks of 512 tokens
                    xt = xstream.tile([128, NKK, QT], bf16, tag="xt")
                    nc.sync.dma_start(out=xt, in_=xTr[:, :, j * QT:(j + 1) * QT])
                    # q^T, k^T : [head_dim part, tok free]
                    for (wsb, dst) in ((wq_sb, qT_sb), (wk_sb, kT_sb)):
                        for h2 in range(HPC):
                            ps = psA.tile([128, QT], f32, tag="qk")
                            for kk in range(NKK):
                                nc.tensor.matmul(
                                    ps,
                                    lhsT=wsb[:, kk, h2 * HD:(h2 + 1) * HD],
                                    rhs=xt[:, kk, :],
                                    start=(kk == 0), stop=(kk == NKK - 1),
                                )
                            nc.any.tensor_copy(
                                dst[:, h2, j * QT:(j + 1) * QT], ps)
                    # v : [tok part, head_dim free]
                    for mm in range(QT // 128):  # 4 token chunks of 128
                        ps = psA.tile([128, HDC], f32, tag="v")
                        for kk in range(NKK):
                            nc.tensor.matmul(
                                ps,
                                lhsT=xt[:, kk, mm * 128:(mm + 1) * 128],
                                rhs=wv_sb[:, kk, :],
                                start=(kk == 0), stop=(kk == NKK - 1),
                            )
                        nc.any.tensor_copy(v_sb[:, j * 4 + mm, :], ps)

            # ---- stage B: causal attention ----------------------------
            from concourse.mybir import ActivationFunctionType as AF
            from concourse.mybir import AluOpType

            with (
                tc.tile_pool(name="psS", bufs=2, space="PSUM") as psS,
                tc.tile_pool(name="psY", bufs=3, space="PSUM") as psY,
                tc.tile_pool(name="psD", bufs=2, space="PSUM") as psD,
                tc.tile_pool(name="psR", bufs=1, space="PSUM") as psR,
            ):
                for b in range(B):
                    for hh in range(HPC):
                        t0 = b * T
                        for iq in range(T // QT):  # 4 q-tiles of 512
                            nkb = (iq + 1) * (QT // KB)
                            y_ps = psY.tile([128, QT], f32, tag="y")
                            d_ps = psD.tile([1, QT], f32, tag="d")

                            def s_block(kb):
                                s_ps = psS.tile([128, QT], f32, tag="s")
                                nc.tensor.matmul(
                                    s_ps,
                                    lhsT=kT_sb[:, hh, t0 + kb * KB:t0 + (kb + 1) * KB],
                                    rhs=qT_sb[:, hh, t0 + iq * QT:t0 + (iq + 1) * QT],
                                    start=True, stop=True,
                                )
                                pt = work.tile([128, QT], bf16, tag="pt")
                                nc.scalar.activation(pt, s_ps, AF.Exp, scale=sqk_sb)
                                if kb >= iq * (QT // KB):
                                    # diagonal block: zero where k_glob > q_glob
                                    nc.gpsimd.affine_select(
                                        out=pt, in_=pt,
                                        pattern=[[1, QT]],
                                        channel_multiplier=-1,
                                        base=iq * QT - kb * KB,
                                        compare_op=AluOpType.is_ge,
                                        fill=0.0,
                                    )
                                return pt

                            # software pipeline: S(kb+1) issued before the
                            # pt(kb)-dependent d/y matmuls so PE never idles
                            # waiting on the Exp of the current block.
                            pts = s_block(0)
                            for kb in range(nkb):
                                pt = pts
                                pts = s_block(kb + 1) if kb + 1 < nkb else None
                                # d[q] += sum_k pt[k, q] (rank-1 on PE)
                                nc.tensor.matmul(
                                    d_ps, lhsT=ones_col, rhs=pt,
                                    start=(kb == 0), stop=(kb == nkb - 1),
                                )
                                nc.tensor.matmul(
                                    y_ps,
                                    lhsT=v_sb[:, b * (T // 128) + kb,
                                              hh * HD:(hh + 1) * HD],
                                    rhs=pt,
                                    start=(kb == 0), stop=(kb == nkb - 1),
                                )
                            # softmax denominator and normalization
                            d_sb = dwork.tile([1, QT], f32, tag="dsb")
                            nc.scalar.copy(d_sb, d_ps)
                            dr = dwork.tile([1, QT], f32, tag="dr")
                            nc.vector.reciprocal(dr, d_sb)
                            r_ps = psR.tile([128, QT], f32, tag="r")
                            # rank-1 broadcast: R[p, q] = s_v*s_o / d[q]
                            nc.tensor.matmul(r_ps, lhsT=svo_sb, rhs=dr,
                                             start=True, stop=True)
                            r_sb = dwork.tile([128, QT], f32, tag="rsb")
                            nc.scalar.copy(r_sb, r_ps)
                            nc.vector.tensor_mul(
                                yT_sb[:, hh, t0 + iq * QT:t0 + (iq + 1) * QT],
                                y_ps, r_sb)

            # ---- stage C: out-projection (partial) --------------------
            with tc.tile_pool(name="psC", bufs=4, space="PSUM") as psC:
                for m in range(BT // 128):  # 32 token chunks
                    for n in range(D // QT):  # 4 output column tiles
                        ps = psC.tile([128, QT], f32, tag="o")
                        for kk in range(HPC):
                            nc.tensor.matmul(
                                ps,
                                lhsT=yT_sb[:, kk, m * 128:(m + 1) * 128],
                                rhs=wo_sb[:, kk, n * QT:(n + 1) * QT],
                                start=(kk == 0), stop=(kk == HPC - 1),
                            )
                        o_sb = outsb.tile([128, QT], bf16, tag="osb")
                        nc.any.tensor_copy(o_sb, ps)
                        nc.sync.dma_start(
                            out=out[m * 128:(m + 1) * 128, n * QT:(n + 1) * QT],
                            in_=o_sb)

    nc.compile()
    return nc


def kernel(x, wq, wk, wv, wo):
    import concourse.bass_utils as bass_utils

    x = np.asarray(x, dtype=np.float32)
    bf16 = ml_dtypes.bfloat16

    if "nc" not in _cache:
        _cache["nc"] = _build_nc()
    nc = _cache["nc"]

    scales = {}
    signs = {}
    for name, w in (("q", wq), ("k", wk), ("v", wv), ("o", wo)):
        w = np.asarray(w, dtype=np.float32)
        scales[name] = max(np.mean(np.abs(w)), 1e-5)
        signs[name] = np.sign(w)

    s_qk = np.float32(scales["q"] * scales["k"] / np.sqrt(HD))
    s_vo = np.float32(scales["v"] * scales["o"])

    xT = np.ascontiguousarray(x.reshape(BT, D).T).astype(bf16)
    scal_qk = np.full((128, 1), s_qk, dtype=np.float32)
    scal_vo = np.full((1, 128), s_vo, dtype=np.float32)

    in_maps = []
    for c in range(NCORES):
        sl = slice(c * HDC, (c + 1) * HDC)
        in_maps.append({
            "xT": xT,
            "wqT": np.ascontiguousarray(signs["q"][sl, :].T).astype(bf16),
            "wkT": np.ascontiguousarray(signs["k"][sl, :].T).astype(bf16),
            "wvT": np.ascontiguousarray(signs["v"][sl, :].T).astype(bf16),
            "woT": np.ascontiguousarray(signs["o"][:, sl].T).astype(bf16),
            "scal_qk": scal_qk,
            "scal_vo": scal_vo,
        })

    res = bass_utils.run_bass_kernel_spmd(nc, in_maps,
                                          core_ids=list(range(NCORES)),
                                          **_cache.get("run_kwargs", {}))
    _cache["last_result"] = res

    acc = np.zeros((BT, D), dtype=np.float32)
    for r in res.results:
        acc += np.asarray(r["out"], dtype=np.float32)
    return acc.reshape(B, T, D)


# revision 9
# speedup vs baseline: 2.9612x; 1.0606x over previous
"""BitAttention (BitNet-style ternary-quantized attention) on 8 Trainium2
NeuronCores.

Sharding: tensor-parallel across heads. 16 heads / 8 cores = 2 heads per
core. Each core computes q/k/v projections for its 2 heads (output-dim
shard), causal attention for those heads, and a partial out-projection
(input-dim shard of wo). Host sums the 8 partial outputs (the all-reduce
of the hint, done at unshard time).

Weight quantization sign(w) * mean(|w|) is separable: the +-1 sign
matrices are exact in bf16 and become matmul operands; the four scalar
scales are folded into the softmax exp scale and the output scale, both
applied on-device in fp32 via tiny input tensors.

Layouts (per core):
  xT   [D, B*T]  x transposed (host), bf16 - moving operand of q/k proj,
                 stationary of v proj.
  q^T,k^T kept [head_dim, tok] in SBUF; V kept [tok, head_dim];
  scores computed transposed S^T = [k-tok, q-tok] so that
  P^T = exp(S^T) feeds the y^T matmul directly (no on-chip transposes
  anywhere). Softmax denominator accumulated on DVE, reduced across
  partitions on GpSimd, broadcast back via a rank-1 matmul.
"""

import numpy as np
import ml_dtypes

B, T, D, H = 2, 2048, 2048, 16
HD = 128  # head dim
NCORES = 8
HPC = H // NCORES  # heads per core = 2
HDC = HPC * HD  # per-core projection width = 256
BT = B * T  # 4096

QT = 512  # q-tile (free dim of S^T / y^T matmuls)
KB = 128  # k-block (partition dim of S^T)

_cache = {}


def _build_nc():
    import concourse.tile as tile
    from concourse import bacc, mybir

    f32 = mybir.dt.float32
    bf16 = mybir.dt.bfloat16

    nc = bacc.Bacc("TRN2", target_bir_lowering=False, debug=False,
                   num_devices=NCORES)

    xT = nc.dram_tensor("xT", [D, BT], bf16, kind="ExternalInput").ap()
    wqT = nc.dram_tensor("wqT", [D, HDC], bf16, kind="ExternalInput").ap()
    wkT = nc.dram_tensor("wkT", [D, HDC], bf16, kind="ExternalInput").ap()
    wvT = nc.dram_tensor("wvT", [D, HDC], bf16, kind="ExternalInput").ap()
    woT = nc.dram_tensor("woT", [HDC, D], bf16, kind="ExternalInput").ap()
    # scal_qk: [128,1] filled with s_q*s_k/sqrt(HD) (exp scale)
    # scal_vo: [1,128] filled with s_v*s_o (folded into 1/d broadcast)
    scal_qk = nc.dram_tensor("scal_qk", [128, 1], f32, kind="ExternalInput").ap()
    scal_vo = nc.dram_tensor("scal_vo", [1, 128], f32, kind="ExternalInput").ap()
    out = nc.dram_tensor("out", [BT, D], bf16, kind="ExternalOutput").ap()

    with tile.TileContext(nc) as tc:
        with (
            tc.tile_pool(name="singles", bufs=1) as singles,
            tc.tile_pool(name="xstream", bufs=2) as xstream,
            tc.tile_pool(name="work", bufs=6) as work,
            tc.tile_pool(name="dwork", bufs=3) as dwork,
            tc.tile_pool(name="outsb", bufs=6) as outsb,
        ):
            # ---- persistent SBUF tensors -------------------------------
            wq_sb = singles.tile([128, D // 128, HDC], bf16, tag="wq")
            wk_sb = singles.tile([128, D // 128, HDC], bf16, tag="wk")
            wv_sb = singles.tile([128, D // 128, HDC], bf16, tag="wv")
            wo_sb = singles.tile([128, HPC, D], bf16, tag="wo")
            sqk_sb = singles.tile([128, 1], f32, tag="sqk")
            svo_sb = singles.tile([1, 128], f32, tag="svo")
            ones_col = singles.tile([128, 1], bf16, tag="ones")
            nc.vector.memset(ones_col, 1.0)
            # per-batch / per-unit splits so later stages can start as soon
            # as their slice of the data is ready (Tile deps are per-tile)
            qT_sb = [singles.tile([128, HPC, T], bf16, tag=f"qT{b}", name=f"qT{b}")
                     for b in range(B)]
            kT_sb = [singles.tile([128, HPC, T], bf16, tag=f"kT{b}", name=f"kT{b}")
                     for b in range(B)]
            v_sb = [singles.tile([128, T // 128, HDC], bf16, tag=f"v{b}", name=f"v{b}")
                    for b in range(B)]
            yT_sb = [[singles.tile([128, T], bf16, tag=f"yT{b}{hh}", name=f"yT{b}{hh}")
                      for hh in range(HPC)] for b in range(B)]

            nc.sync.dma_start(out=wq_sb, in_=wqT.rearrange("(c p) m -> p c m", p=128))
            nc.sync.dma_start(out=wk_sb, in_=wkT.rearrange("(c p) m -> p c m", p=128))
            nc.sync.dma_start(out=wv_sb, in_=wvT.rearrange("(c p) m -> p c m", p=128))
            nc.sync.dma_start(out=wo_sb, in_=woT.rearrange("(c p) m -> p c m", p=128))
            nc.sync.dma_start(out=sqk_sb, in_=scal_qk)
            nc.sync.dma_start(out=svo_sb, in_=scal_vo)

            # ---- stage A: projections ---------------------------------
            NKK = D // 128  # 16 contraction chunks
            xTr = xT.rearrange("(c p) n -> p c n", p=128)
            with tc.tile_pool(name="psA", bufs=2, space="PSUM") as psA:
                for j in range(BT // QT):  # 8 column blocks of 512 tokens
                    b, jb = divmod(j, T // QT)
                    xt = xstream.tile([128, NKK, QT], bf16, tag="xt")
                    nc.sync.dma_start(out=xt, in_=xTr[:, :, j * QT:(j + 1) * QT])
                    # q^T, k^T : [head_dim part, tok free]
                    for (wsb, dst) in ((wq_sb, qT_sb[b]), (wk_sb, kT_sb[b])):
                        for h2 in range(HPC):
                            ps = psA.tile([128, QT], f32, tag="qk")
                            for kk in range(NKK):
                                nc.tensor.matmul(
                                    ps,
                                    lhsT=wsb[:, kk, h2 * HD:(h2 + 1) * HD],
                                    rhs=xt[:, kk, :],
                                    start=(kk == 0), stop=(kk == NKK - 1),
                                )
                            nc.any.tensor_copy(
                                dst[:, h2, jb * QT:(jb + 1) * QT], ps)
                    # v : [tok part, head_dim free]
                    for mm in range(QT // 128):  # 4 token chunks of 128
                        ps = psA.tile([128, HDC], f32, tag="v")
                        for kk in range(NKK):
                            nc.tensor.matmul(
                                ps,
                                lhsT=xt[:, kk, mm * 128:(mm + 1) * 128],
                                rhs=wv_sb[:, kk, :],
                                start=(kk == 0), stop=(kk == NKK - 1),
                            )
                        nc.any.tensor_copy(v_sb[b][:, jb * 4 + mm, :], ps)

            # ---- stage B: causal attention ----------------------------
            from concourse.mybir import ActivationFunctionType as AF
            from concourse.mybir import AluOpType

            LOOKAHEAD = 2
            with (
                tc.tile_pool(name="psS", bufs=3, space="PSUM") as psS,
                tc.tile_pool(name="psY", bufs=3, space="PSUM") as psY,
                tc.tile_pool(name="psDR", bufs=2, space="PSUM") as psDR,
            ):
                for b in range(B):
                    for hh in range(HPC):
                        for iq in range(T // QT):  # 4 q-tiles of 512
                            nkb = (iq + 1) * (QT // KB)
                            y_ps = psY.tile([128, QT], f32, tag="y")
                            d_ps = psDR.tile([1, QT], f32, tag="dr")

                            def s_block(kb):
                                s_ps = psS.tile([128, QT], f32, tag="s")
                                nc.tensor.matmul(
                                    s_ps,
                                    lhsT=kT_sb[b][:, hh, kb * KB:(kb + 1) * KB],
                                    rhs=qT_sb[b][:, hh, iq * QT:(iq + 1) * QT],
                                    start=True, stop=True,
                                )
                                pt = work.tile([128, QT], bf16, tag="pt")
                                nc.scalar.activation(pt, s_ps, AF.Exp, scale=sqk_sb)
                                if kb >= iq * (QT // KB):
                                    # diagonal block: zero where k_glob > q_glob
                                    nc.gpsimd.affine_select(
                                        out=pt, in_=pt,
                                        pattern=[[1, QT]],
                                        channel_multiplier=-1,
                                        base=iq * QT - kb * KB,
                                        compare_op=AluOpType.is_ge,
                                        fill=0.0,
                                    )
                                return pt

                            # software pipeline: S(kb+L) issued before the
                            # pt(kb)-dependent d/y matmuls so PE keeps ahead
                            # of the Exp (ScalarE) latency.
                            pts = [s_block(kb) for kb in range(min(LOOKAHEAD, nkb))]
                            for kb in range(nkb):
                                pt = pts[kb]
                                if kb + LOOKAHEAD < nkb:
                                    pts.append(s_block(kb + LOOKAHEAD))
                                # d[q] += sum_k pt[k, q] (rank-1 on PE)
                                nc.tensor.matmul(
                                    d_ps, lhsT=ones_col, rhs=pt,
                                    start=(kb == 0), stop=(kb == nkb - 1),
                                )
                                nc.tensor.matmul(
                                    y_ps,
                                    lhsT=v_sb[b][:, kb, hh * HD:(hh + 1) * HD],
                                    rhs=pt,
                                    start=(kb == 0), stop=(kb == nkb - 1),
                                )
                            # softmax denominator and normalization
                            d_sb = dwork.tile([1, QT], f32, tag="dsb")
                            nc.vector.tensor_copy(d_sb, d_ps)
                            dr = dwork.tile([1, QT], f32, tag="dr")
                            nc.vector.reciprocal(dr, d_sb)
                            r_ps = psDR.tile([128, QT], f32, tag="dr")
                            # rank-1 broadcast: R[p, q] = s_v*s_o / d[q]
                            nc.tensor.matmul(r_ps, lhsT=svo_sb, rhs=dr,
                                             start=True, stop=True)
                            r_sb = dwork.tile([128, QT], f32, tag="rsb")
                            nc.vector.tensor_copy(r_sb, r_ps)
                            nc.vector.tensor_mul(
                                yT_sb[b][hh][:, iq * QT:(iq + 1) * QT],
                                y_ps, r_sb)

            # ---- stage C: out-projection (partial) --------------------
            with tc.tile_pool(name="psC", bufs=4, space="PSUM") as psC:
                for m in range(BT // 128):  # 32 token chunks
                    b, mb = divmod(m, T // 128)
                    for n in range(D // QT):  # 4 output column tiles
                        ps = psC.tile([128, QT], f32, tag="o")
                        for kk in range(HPC):
                            nc.tensor.matmul(
                                ps,
                                lhsT=yT_sb[b][kk][:, mb * 128:(mb + 1) * 128],
                                rhs=wo_sb[:, kk, n * QT:(n + 1) * QT],
                                start=(kk == 0), stop=(kk == HPC - 1),
                            )
                        o_sb = outsb.tile([128, QT], bf16, tag="osb")
                        nc.vector.tensor_copy(o_sb, ps)
                        nc.sync.dma_start(
                            out=out[m * 128:(m + 1) * 128, n * QT:(n + 1) * QT],
                            in_=o_sb)

    nc.compile()
    return nc


def kernel(x, wq, wk, wv, wo):
    import concourse.bass_utils as bass_utils

    x = np.asarray(x, dtype=np.float32)
    bf16 = ml_dtypes.bfloat16

    if "nc" not in _cache:
        _cache["nc"] = _build_nc()
    nc = _cache["nc"]

    scales = {}
    signs = {}
    for name, w in (("q", wq), ("k", wk), ("v", wv), ("o", wo)):
        w = np.asarray(w, dtype=np.float32)
        scales[name] = max(np.mean(np.abs(w)), 1e-5)
        signs[name] = np.sign(w)

    s_qk = np.float32(scales["q"] * scales["k"] / np.sqrt(HD))
    s_vo = np.float32(scales["v"] * scales["o"])

    xT = np.ascontiguousarray(x.reshape(BT, D).T).astype(bf16)
    scal_qk = np.full((128, 1), s_qk, dtype=np.float32)
    scal_vo = np.full((1, 128), s_vo, dtype=np.float32)

    in_maps = []
    for c in range(NCORES):
        sl = slice(c * HDC, (c + 1) * HDC)
        in_maps.append({
            "xT": xT,
            "wqT": np.ascontiguousarray(signs["q"][sl, :].T).astype(bf16),
            "wkT": np.ascontiguousarray(signs["k"][sl, :].T).astype(bf16),
            "wvT": np.ascontiguousarray(signs["v"][sl, :].T).astype(bf16),
            "woT": np.ascontiguousarray(signs["o"][:, sl].T).astype(bf16),
            "scal_qk": scal_qk,
            "scal_vo": scal_vo,
        })

    res = bass_utils.run_bass_kernel_spmd(nc, in_maps,
                                          core_ids=list(range(NCORES)),
                                          **_cache.get("run_kwargs", {}))
    _cache["last_result"] = res

    acc = np.zeros((BT, D), dtype=np.float32)
    for r in res.results:
        acc += np.asarray(r["out"], dtype=np.float32)
    return acc.reshape(B, T, D)


# revision 12
# speedup vs baseline: 3.3459x; 1.1299x over previous
"""BitAttention (BitNet-style ternary-quantized attention) on 8 Trainium2
NeuronCores.

Sharding: tensor-parallel across heads. 16 heads / 8 cores = 2 heads per
core. Each core computes q/k/v projections for its 2 heads (output-dim
shard), causal attention for those heads, and a partial out-projection
(input-dim shard of wo). Host sums the 8 partial outputs (the all-reduce
of the hint, done at unshard time).

Weight quantization sign(w) * mean(|w|) is separable: the +-1 sign
matrices are exact in bf16 and become matmul operands; the four scalar
scales are folded into the softmax exp scale and the output scale, both
applied on-device in fp32 via tiny input tensors.

Layouts (per core):
  xT   [D, B*T]  x transposed (host), bf16 - moving operand of q/k proj,
                 stationary of v proj.
  q^T,k^T kept [head_dim, tok] in SBUF; V kept [tok, head_dim];
  scores computed transposed S^T = [k-tok, q-tok] so that
  P^T = exp(S^T) feeds the y^T matmul directly (no on-chip transposes
  anywhere). Softmax denominator accumulated on DVE, reduced across
  partitions on GpSimd, broadcast back via a rank-1 matmul.
"""

import numpy as np
import ml_dtypes

B, T, D, H = 2, 2048, 2048, 16
HD = 128  # head dim
NCORES = 8
HPC = H // NCORES  # heads per core = 2
HDC = HPC * HD  # per-core projection width = 256
BT = B * T  # 4096

QT = 512  # q-tile (free dim of S^T / y^T matmuls)
KB = 128  # k-block (partition dim of S^T)

_cache = {}


def _build_nc():
    import concourse.tile as tile
    from concourse import bacc, mybir

    f32 = mybir.dt.float32
    bf16 = mybir.dt.bfloat16

    nc = bacc.Bacc("TRN2", target_bir_lowering=False, debug=False,
                   num_devices=NCORES)

    xT = nc.dram_tensor("xT", [D, BT], bf16, kind="ExternalInput").ap()
    wqT = nc.dram_tensor("wqT", [D, HDC], bf16, kind="ExternalInput").ap()
    wkT = nc.dram_tensor("wkT", [D, HDC], bf16, kind="ExternalInput").ap()
    wvT = nc.dram_tensor("wvT", [D, HDC], bf16, kind="ExternalInput").ap()
    woT = nc.dram_tensor("woT", [HDC, D], bf16, kind="ExternalInput").ap()
    # scal_qk: [128,1] filled with s_q*s_k/sqrt(HD) (folded into q^T)
    # scal_vo: [1,1] filled with 1/(s_v*s_o) (folded into softmax denom)
    scal_qk = nc.dram_tensor("scal_qk", [128, 1], f32, kind="ExternalInput").ap()
    scal_vo = nc.dram_tensor("scal_vo", [1, 1], f32, kind="ExternalInput").ap()
    out = nc.dram_tensor("out", [BT, D], bf16, kind="ExternalOutput").ap()

    with tile.TileContext(nc) as tc:
        with (
            tc.tile_pool(name="singles", bufs=1) as singles,
            tc.tile_pool(name="xstream", bufs=2) as xstream,
            tc.tile_pool(name="work", bufs=6) as work,
            tc.tile_pool(name="dwork", bufs=3) as dwork,
            tc.tile_pool(name="outsb", bufs=6) as outsb,
        ):
            # ---- persistent SBUF tensors -------------------------------
            wq_sb = singles.tile([128, D // 128, HDC], bf16, tag="wq")
            wk_sb = singles.tile([128, D // 128, HDC], bf16, tag="wk")
            wv_sb = singles.tile([128, D // 128, HDC], bf16, tag="wv")
            wo_sb = singles.tile([128, HPC, D], bf16, tag="wo")
            sqk_sb = singles.tile([128, 1], f32, tag="sqk")
            svo_sb = singles.tile([1, 1], f32, tag="svo")
            ones_col = singles.tile([128, 1], bf16, tag="ones")
            nc.vector.memset(ones_col, 1.0)
            # per-batch / per-unit splits so later stages can start as soon
            # as their slice of the data is ready (Tile deps are per-tile)
            qT_sb = [singles.tile([128, HPC, T], bf16, tag=f"qT{b}", name=f"qT{b}")
                     for b in range(B)]
            kT_sb = [singles.tile([128, HPC, T], bf16, tag=f"kT{b}", name=f"kT{b}")
                     for b in range(B)]
            v_sb = [singles.tile([128, T // 128, HDC], bf16, tag=f"v{b}", name=f"v{b}")
                    for b in range(B)]
            yT_sb = [[singles.tile([128, T], bf16, tag=f"yT{b}{hh}", name=f"yT{b}{hh}")
                      for hh in range(HPC)] for b in range(B)]

            nc.gpsimd.dma_start(out=wq_sb, in_=wqT.rearrange("(c p) m -> p c m", p=128))
            nc.gpsimd.dma_start(out=wk_sb, in_=wkT.rearrange("(c p) m -> p c m", p=128))
            nc.gpsimd.dma_start(out=wv_sb, in_=wvT.rearrange("(c p) m -> p c m", p=128))
            nc.gpsimd.dma_start(out=wo_sb, in_=woT.rearrange("(c p) m -> p c m", p=128))
            nc.gpsimd.dma_start(out=sqk_sb, in_=scal_qk)
            nc.gpsimd.dma_start(out=svo_sb, in_=scal_vo)

            # ---- stage A: projections ---------------------------------
            NKK = D // 128  # 16 contraction chunks
            xTr = xT.rearrange("(c p) n -> p c n", p=128)
            with tc.tile_pool(name="psA", bufs=2, space="PSUM") as psA:
                for j in range(BT // QT):  # 8 column blocks of 512 tokens
                    b, jb = divmod(j, T // QT)
                    xt = xstream.tile([128, NKK, QT], bf16, tag="xt")
                    nc.sync.dma_start(out=xt, in_=xTr[:, :, j * QT:(j + 1) * QT])
                    # q^T, k^T : [head_dim part, tok free]
                    for (wsb, dst) in ((wq_sb, qT_sb[b]), (wk_sb, kT_sb[b])):
                        for h2 in range(HPC):
                            ps = psA.tile([128, QT], f32, tag="qk")
                            for kk in range(NKK):
                                nc.tensor.matmul(
                                    ps,
                                    lhsT=wsb[:, kk, h2 * HD:(h2 + 1) * HD],
                                    rhs=xt[:, kk, :],
                                    start=(kk == 0), stop=(kk == NKK - 1),
                                )
                            if wsb is wq_sb:
                                # fold exp scale s_q*s_k/sqrt(hd) into q^T
                                nc.vector.tensor_scalar_mul(
                                    dst[:, h2, jb * QT:(jb + 1) * QT], ps, sqk_sb)
                            else:
                                nc.any.tensor_copy(
                                    dst[:, h2, jb * QT:(jb + 1) * QT], ps)
                    # v : [tok part, head_dim free]
                    for mm in range(QT // 128):  # 4 token chunks of 128
                        ps = psA.tile([128, HDC], f32, tag="v")
                        for kk in range(NKK):
                            nc.tensor.matmul(
                                ps,
                                lhsT=xt[:, kk, mm * 128:(mm + 1) * 128],
                                rhs=wv_sb[:, kk, :],
                                start=(kk == 0), stop=(kk == NKK - 1),
                            )
                        nc.any.tensor_copy(v_sb[b][:, jb * 4 + mm, :], ps)

            # ---- stage B: causal attention ----------------------------
            from concourse.mybir import ActivationFunctionType as AF
            from concourse.mybir import AluOpType

            LOOKAHEAD = 1  # in pairs of k-blocks
            with (
                tc.tile_pool(name="psS", bufs=2, space="PSUM") as psS,
                tc.tile_pool(name="psY", bufs=2, space="PSUM") as psY,
                tc.tile_pool(name="psD", bufs=2, space="PSUM") as psD,
            ):
                for b in range(B):
                    for hh in range(HPC):
                        for iq in range(T // QT):  # 4 q-tiles of 512
                            npair = (iq + 1) * (QT // KB) // 2
                            nkb = npair * 2
                            y_ps = psY.tile([128, QT], f32, tag="y")
                            d_ps = psD.tile([1, QT], f32, tag="d")

                            def s_pair(pr):
                                # two k-blocks -> one 2-bank PSUM tile, one
                                # Exp op, one (paired) causal mask op
                                s_ps = psS.tile([128, 2, QT], f32, tag="s")
                                for j in range(2):
                                    kb = pr * 2 + j
                                    nc.tensor.matmul(
                                        s_ps[:, j, :],
                                        lhsT=kT_sb[b][:, hh, kb * KB:(kb + 1) * KB],
                                        rhs=qT_sb[b][:, hh, iq * QT:(iq + 1) * QT],
                                        start=True, stop=True,
                                    )
                                pt = work.tile([128, 2, QT], bf16, tag="pt")
                                nc.scalar.activation(pt, s_ps, AF.Exp)
                                if (pr + 1) * 2 > iq * (QT // KB):
                                    # pair touches the diagonal: zero k>q
                                    nc.gpsimd.affine_select(
                                        out=pt, in_=pt,
                                        pattern=[[-KB, 2], [1, QT]],
                                        channel_multiplier=-1,
                                        base=iq * QT - pr * 2 * KB,
                                        compare_op=AluOpType.is_ge,
                                        fill=0.0,
                                    )
                                return pt

                            pts = [s_pair(p) for p in range(min(LOOKAHEAD, npair))]
                            for pr in range(npair):
                                pt = pts[pr]
                                if pr + LOOKAHEAD < npair:
                                    pts.append(s_pair(pr + LOOKAHEAD))
                                for j in range(2):
                                    kb = pr * 2 + j
                                    # d[q] += sum_k pt[k, q] (rank-1 on PE)
                                    nc.tensor.matmul(
                                        d_ps, lhsT=ones_col, rhs=pt[:, j, :],
                                        start=(kb == 0), stop=(kb == nkb - 1),
                                    )
                                    nc.tensor.matmul(
                                        y_ps,
                                        lhsT=v_sb[b][:, kb, hh * HD:(hh + 1) * HD],
                                        rhs=pt[:, j, :],
                                        start=(kb == 0), stop=(kb == nkb - 1),
                                    )
                            # softmax denominator and normalization
                            # (DVE/GpSimd only - no PE op, so the next
                            # q-tile's matmuls proceed without stalling)
                            d_sb = dwork.tile([1, QT], f32, tag="dsb")
                            nc.vector.tensor_scalar_mul(d_sb, d_ps, svo_sb)
                            dr = dwork.tile([1, QT], f32, tag="dr")
                            nc.vector.reciprocal(dr, d_sb)
                            r_sb = dwork.tile([128, QT], f32, tag="rsb")
                            nc.gpsimd.partition_broadcast(r_sb, dr)
                            nc.vector.tensor_mul(
                                yT_sb[b][hh][:, iq * QT:(iq + 1) * QT],
                                y_ps, r_sb)

            # ---- stage C: out-projection (partial) --------------------
            with tc.tile_pool(name="psC", bufs=4, space="PSUM") as psC:
                for m in range(BT // 128):  # 32 token chunks
                    b, mb = divmod(m, T // 128)
                    for n in range(D // QT):  # 4 output column tiles
                        ps = psC.tile([128, QT], f32, tag="o")
                        for kk in range(HPC):
                            nc.tensor.matmul(
                                ps,
                                lhsT=yT_sb[b][kk][:, mb * 128:(mb + 1) * 128],
                                rhs=wo_sb[:, kk, n * QT:(n + 1) * QT],
                                start=(kk == 0), stop=(kk == HPC - 1),
                            )
                        o_sb = outsb.tile([128, QT], bf16, tag="osb")
                        nc.vector.tensor_copy(o_sb, ps)
                        nc.sync.dma_start(
                            out=out[m * 128:(m + 1) * 128, n * QT:(n + 1) * QT],
                            in_=o_sb)

    nc.compile()
    return nc


def kernel(x, wq, wk, wv, wo):
    import concourse.bass_utils as bass_utils

    x = np.asarray(x, dtype=np.float32)
    bf16 = ml_dtypes.bfloat16

    if "nc" not in _cache:
        _cache["nc"] = _build_nc()
    nc = _cache["nc"]

    scales = {}
    signs = {}
    for name, w in (("q", wq), ("k", wk), ("v", wv), ("o", wo)):
        w = np.asarray(w, dtype=np.float32)
        scales[name] = max(np.mean(np.abs(w)), 1e-5)
        signs[name] = np.sign(w)

    s_qk = np.float32(scales["q"] * scales["k"] / np.sqrt(HD))
    s_vo = np.float32(1.0 / (scales["v"] * scales["o"]))

    xT = np.ascontiguousarray(x.reshape(BT, D).T).astype(bf16)
    scal_qk = np.full((128, 1), s_qk, dtype=np.float32)
    scal_vo = np.full((1, 1), s_vo, dtype=np.float32)

    in_maps = []
    for c in range(NCORES):
        sl = slice(c * HDC, (c + 1) * HDC)
        in_maps.append({
            "xT": xT,
            "wqT": np.ascontiguousarray(signs["q"][sl, :].T).astype(bf16),
            "wkT": np.ascontiguousarray(signs["k"][sl, :].T).astype(bf16),
            "wvT": np.ascontiguousarray(signs["v"][sl, :].T).astype(bf16),
            "woT": np.ascontiguousarray(signs["o"][:, sl].T).astype(bf16),
            "scal_qk": scal_qk,
            "scal_vo": scal_vo,
        })

    res = bass_utils.run_bass_kernel_spmd(nc, in_maps,
                                          core_ids=list(range(NCORES)),
                                          **_cache.get("run_kwargs", {}))
    _cache["last_result"] = res

    acc = np.zeros((BT, D), dtype=np.float32)
    for r in res.results:
        acc += np.asarray(r["out"], dtype=np.float32)
    return acc.reshape(B, T, D)


# revision 13
# speedup vs baseline: 3.5901x; 1.0730x over previous
"""BitAttention (BitNet-style ternary-quantized attention) on 8 Trainium2
NeuronCores.

Sharding: tensor-parallel across heads. 16 heads / 8 cores = 2 heads per
core. Each core computes q/k/v projections for its 2 heads (output-dim
shard), causal attention for those heads, and a partial out-projection
(input-dim shard of wo). Host sums the 8 partial outputs (the all-reduce
of the hint, done at unshard time).

Weight quantization sign(w) * mean(|w|) is separable: the +-1 sign
matrices are exact in bf16 and become matmul operands; the four scalar
scales are folded into the softmax exp scale and the output scale, both
applied on-device in fp32 via tiny input tensors.

Layouts (per core):
  xT   [D, B*T]  x transposed (host), bf16 - moving operand of q/k proj,
                 stationary of v proj.
  q^T,k^T kept [head_dim, tok] in SBUF; V kept [tok, head_dim];
  scores computed transposed S^T = [k-tok, q-tok] so that
  P^T = exp(S^T) feeds the y^T matmul directly (no on-chip transposes
  anywhere). Softmax denominator accumulated on DVE, reduced across
  partitions on GpSimd, broadcast back via a rank-1 matmul.
"""

import numpy as np
import ml_dtypes

B, T, D, H = 2, 2048, 2048, 16
HD = 128  # head dim
NCORES = 8
HPC = H // NCORES  # heads per core = 2
HDC = HPC * HD  # per-core projection width = 256
BT = B * T  # 4096

QT = 512  # q-tile (free dim of S^T / y^T matmuls)
KB = 128  # k-block (partition dim of S^T)

_cache = {}


def _build_nc():
    import concourse.tile as tile
    from concourse import bacc, mybir

    f32 = mybir.dt.float32
    bf16 = mybir.dt.bfloat16

    nc = bacc.Bacc("TRN2", target_bir_lowering=False, debug=False,
                   num_devices=NCORES)

    xT = nc.dram_tensor("xT", [D, BT], bf16, kind="ExternalInput").ap()
    wqT = nc.dram_tensor("wqT", [D, HDC], bf16, kind="ExternalInput").ap()
    wkT = nc.dram_tensor("wkT", [D, HDC], bf16, kind="ExternalInput").ap()
    wvT = nc.dram_tensor("wvT", [D, HDC], bf16, kind="ExternalInput").ap()
    woT = nc.dram_tensor("woT", [HDC, D], bf16, kind="ExternalInput").ap()
    # scal_qk: [128,1] filled with s_q*s_k/sqrt(HD) (folded into q^T)
    # scal_vo: [1,1] filled with 1/(s_v*s_o) (folded into softmax denom)
    scal_qk = nc.dram_tensor("scal_qk", [128, 1], f32, kind="ExternalInput").ap()
    scal_vo = nc.dram_tensor("scal_vo", [1, 1], f32, kind="ExternalInput").ap()
    out = nc.dram_tensor("out", [BT, D], bf16, kind="ExternalOutput").ap()

    with tile.TileContext(nc) as tc:
        with (
            tc.tile_pool(name="singles", bufs=1) as singles,
            tc.tile_pool(name="xstream", bufs=2) as xstream,
            tc.tile_pool(name="work", bufs=6) as work,
            tc.tile_pool(name="dwork", bufs=3) as dwork,
            tc.tile_pool(name="outsb", bufs=6) as outsb,
        ):
            # ---- persistent SBUF tensors -------------------------------
            wq_sb = singles.tile([128, D // 128, HDC], bf16, tag="wq")
            wk_sb = singles.tile([128, D // 128, HDC], bf16, tag="wk")
            wv_sb = singles.tile([128, D // 128, HDC], bf16, tag="wv")
            wo_sb = singles.tile([128, HPC, D], bf16, tag="wo")
            sqk_sb = singles.tile([128, 1], f32, tag="sqk")
            svo_sb = singles.tile([1, 1], f32, tag="svo")
            ones_col = singles.tile([128, 1], bf16, tag="ones")
            nc.vector.memset(ones_col, 1.0)
            # 0/1 causal masks for the two diagonal pair offsets
            from concourse.mybir import AluOpType as _Alu
            mask_sb = []
            for mi in range(2):
                mk = singles.tile([128, 2, QT], bf16, tag=f"mask{mi}",
                                  name=f"mask{mi}")
                nc.gpsimd.memset(mk, 1.0)
                nc.gpsimd.affine_select(
                    out=mk, in_=mk,
                    pattern=[[-KB, 2], [1, QT]],
                    channel_multiplier=-1,
                    base=-mi * 2 * KB,
                    compare_op=_Alu.is_ge,
                    fill=0.0,
                )
                mask_sb.append(mk)
            # per-batch / per-unit splits so later stages can start as soon
            # as their slice of the data is ready (Tile deps are per-tile)
            qT_sb = [singles.tile([128, HPC, T], bf16, tag=f"qT{b}", name=f"qT{b}")
                     for b in range(B)]
            kT_sb = [singles.tile([128, HPC, T], bf16, tag=f"kT{b}", name=f"kT{b}")
                     for b in range(B)]
            v_sb = [singles.tile([128, T // 128, HDC], bf16, tag=f"v{b}", name=f"v{b}")
                    for b in range(B)]
            yT_sb = [[singles.tile([128, T], bf16, tag=f"yT{b}{hh}", name=f"yT{b}{hh}")
                      for hh in range(HPC)] for b in range(B)]

            nc.gpsimd.dma_start(out=wq_sb, in_=wqT.rearrange("(c p) m -> p c m", p=128))
            nc.gpsimd.dma_start(out=wk_sb, in_=wkT.rearrange("(c p) m -> p c m", p=128))
            nc.gpsimd.dma_start(out=wv_sb, in_=wvT.rearrange("(c p) m -> p c m", p=128))
            nc.gpsimd.dma_start(out=wo_sb, in_=woT.rearrange("(c p) m -> p c m", p=128))
            nc.gpsimd.dma_start(out=sqk_sb, in_=scal_qk)
            nc.gpsimd.dma_start(out=svo_sb, in_=scal_vo)

            # ---- stage A: projections ---------------------------------
            NKK = D // 128  # 16 contraction chunks
            xTr = xT.rearrange("(c p) n -> p c n", p=128)
            with tc.tile_pool(name="psA", bufs=2, space="PSUM") as psA:
                for j in range(BT // QT):  # 8 column blocks of 512 tokens
                    b, jb = divmod(j, T // QT)
                    xt = xstream.tile([128, NKK, QT], bf16, tag="xt")
                    nc.sync.dma_start(out=xt, in_=xTr[:, :, j * QT:(j + 1) * QT])
                    # q^T, k^T : [head_dim part, tok free]
                    for (wsb, dst) in ((wq_sb, qT_sb[b]), (wk_sb, kT_sb[b])):
                        for h2 in range(HPC):
                            ps = psA.tile([128, QT], f32, tag="qk")
                            for kk in range(NKK):
                                nc.tensor.matmul(
                                    ps,
                                    lhsT=wsb[:, kk, h2 * HD:(h2 + 1) * HD],
                                    rhs=xt[:, kk, :],
                                    start=(kk == 0), stop=(kk == NKK - 1),
                                )
                            if wsb is wq_sb:
                                # fold exp scale s_q*s_k/sqrt(hd) into q^T
                                nc.vector.tensor_scalar_mul(
                                    dst[:, h2, jb * QT:(jb + 1) * QT], ps, sqk_sb)
                            else:
                                nc.any.tensor_copy(
                                    dst[:, h2, jb * QT:(jb + 1) * QT], ps)
                    # v : [tok part, head_dim free]
                    for mm in range(QT // 128):  # 4 token chunks of 128
                        ps = psA.tile([128, HDC], f32, tag="v")
                        for kk in range(NKK):
                            nc.tensor.matmul(
                                ps,
                                lhsT=xt[:, kk, mm * 128:(mm + 1) * 128],
                                rhs=wv_sb[:, kk, :],
                                start=(kk == 0), stop=(kk == NKK - 1),
                            )
                        nc.any.tensor_copy(v_sb[b][:, jb * 4 + mm, :], ps)

            # ---- stage B: causal attention ----------------------------
            from concourse.mybir import ActivationFunctionType as AF
            from concourse.mybir import AluOpType

            LOOKAHEAD = 1  # in pairs of k-blocks
            with (
                tc.tile_pool(name="psS", bufs=2, space="PSUM") as psS,
                tc.tile_pool(name="psY", bufs=3, space="PSUM") as psY,
                tc.tile_pool(name="psD", bufs=1, space="PSUM") as psD,
            ):
                for b in range(B):
                    for hh in range(HPC):
                        for iq in range(T // QT):  # 4 q-tiles of 512
                            npair = (iq + 1) * (QT // KB) // 2
                            nkb = npair * 2
                            y_ps = psY.tile([128, QT], f32, tag="y")
                            d_ps = psD.tile([1, QT], f32, tag="d")

                            def s_pair(pr):
                                # two k-blocks -> one 2-bank PSUM tile, one
                                # Exp op, one (paired) causal mask op
                                s_ps = psS.tile([128, 2, QT], f32, tag="s")
                                for j in range(2):
                                    kb = pr * 2 + j
                                    nc.tensor.matmul(
                                        s_ps[:, j, :],
                                        lhsT=kT_sb[b][:, hh, kb * KB:(kb + 1) * KB],
                                        rhs=qT_sb[b][:, hh, iq * QT:(iq + 1) * QT],
                                        start=True, stop=True,
                                    )
                                pt = work.tile([128, 2, QT], bf16, tag="pt")
                                nc.scalar.activation(pt, s_ps, AF.Exp)
                                if pr >= 2 * iq:
                                    # pair touches the diagonal: zero k>q
                                    nc.vector.tensor_mul(pt, pt,
                                                         mask_sb[pr - 2 * iq])
                                return pt

                            pts = [s_pair(p) for p in range(min(LOOKAHEAD, npair))]
                            for pr in range(npair):
                                pt = pts[pr]
                                if pr + LOOKAHEAD < npair:
                                    pts.append(s_pair(pr + LOOKAHEAD))
                                for j in range(2):
                                    kb = pr * 2 + j
                                    # d[q] += sum_k pt[k, q] (rank-1 on PE)
                                    nc.tensor.matmul(
                                        d_ps, lhsT=ones_col, rhs=pt[:, j, :],
                                        start=(kb == 0), stop=(kb == nkb - 1),
                                    )
                                    nc.tensor.matmul(
                                        y_ps,
                                        lhsT=v_sb[b][:, kb, hh * HD:(hh + 1) * HD],
                                        rhs=pt[:, j, :],
                                        start=(kb == 0), stop=(kb == nkb - 1),
                                    )
                            # softmax denominator and normalization
                            # (DVE/GpSimd only - no PE op, so the next
                            # q-tile's matmuls proceed without stalling)
                            d_sb = dwork.tile([1, QT], f32, tag="dsb")
                            nc.vector.tensor_scalar_mul(d_sb, d_ps, svo_sb)
                            dr = dwork.tile([1, QT], f32, tag="dr")
                            nc.vector.reciprocal_approx_fast(dr, d_sb)
                            r_sb = dwork.tile([128, QT], f32, tag="rsb")
                            nc.gpsimd.partition_broadcast(r_sb, dr)
                            nc.vector.tensor_mul(
                                yT_sb[b][hh][:, iq * QT:(iq + 1) * QT],
                                y_ps, r_sb)

            # ---- stage C: out-projection (partial) --------------------
            with tc.tile_pool(name="psC", bufs=4, space="PSUM") as psC:
                for m in range(BT // 128):  # 32 token chunks
                    b, mb = divmod(m, T // 128)
                    for n in range(D // QT):  # 4 output column tiles
                        ps = psC.tile([128, QT], f32, tag="o")
                        for kk in range(HPC):
                            nc.tensor.matmul(
                                ps,
                                lhsT=yT_sb[b][kk][:, mb * 128:(mb + 1) * 128],
                                rhs=wo_sb[:, kk, n * QT:(n + 1) * QT],
                                start=(kk == 0), stop=(kk == HPC - 1),
                            )
                        o_sb = outsb.tile([128, QT], bf16, tag="osb")
                        nc.vector.tensor_copy(o_sb, ps)
                        nc.sync.dma_start(
                            out=out[m * 128:(m + 1) * 128, n * QT:(n + 1) * QT],
                            in_=o_sb)

    nc.compile()
    return nc


def kernel(x, wq, wk, wv, wo):
    import concourse.bass_utils as bass_utils

    x = np.asarray(x, dtype=np.float32)
    bf16 = ml_dtypes.bfloat16

    if "nc" not in _cache:
        _cache["nc"] = _build_nc()
    nc = _cache["nc"]

    scales = {}
    signs = {}
    for name, w in (("q", wq), ("k", wk), ("v", wv), ("o", wo)):
        w = np.asarray(w, dtype=np.float32)
        scales[name] = max(np.mean(np.abs(w)), 1e-5)
        signs[name] = np.sign(w)

    s_qk = np.float32(scales["q"] * scales["k"] / np.sqrt(HD))
    s_vo = np.float32(1.0 / (scales["v"] * scales["o"]))

    xT = np.ascontiguousarray(x.reshape(BT, D).T).astype(bf16)
    scal_qk = np.full((128, 1), s_qk, dtype=np.float32)
    scal_vo = np.full((1, 1), s_vo, dtype=np.float32)

    in_maps = []
    for c in range(NCORES):
        sl = slice(c * HDC, (c + 1) * HDC)
        in_maps.append({
            "xT": xT,
            "wqT": np.ascontiguousarray(signs["q"][sl, :].T).astype(bf16),
            "wkT": np.ascontiguousarray(signs["k"][sl, :].T).astype(bf16),
            "wvT": np.ascontiguousarray(signs["v"][sl, :].T).astype(bf16),
            "woT": np.ascontiguousarray(signs["o"][:, sl].T).astype(bf16),
            "scal_qk": scal_qk,
            "scal_vo": scal_vo,
        })

    res = bass_utils.run_bass_kernel_spmd(nc, in_maps,
                                          core_ids=list(range(NCORES)),
                                          **_cache.get("run_kwargs", {}))
    _cache["last_result"] = res

    acc = np.zeros((BT, D), dtype=np.float32)
    for r in res.results:
        acc += np.asarray(r["out"], dtype=np.float32)
    return acc.reshape(B, T, D)


# revision 14
# speedup vs baseline: 3.5911x; 1.0003x over previous
"""BitAttention (BitNet-style ternary-quantized attention) on 8 Trainium2
NeuronCores.

Sharding: tensor-parallel across heads. 16 heads / 8 cores = 2 heads per
core. Each core computes q/k/v projections for its 2 heads (output-dim
shard), causal attention for those heads, and a partial out-projection
(input-dim shard of wo). Host sums the 8 partial outputs (the all-reduce
of the hint, done at unshard time).

Weight quantization sign(w) * mean(|w|) is separable: the +-1 sign
matrices are exact in bf16 and become matmul operands; the four scalar
scales are folded into the softmax exp scale and the output scale, both
applied on-device in fp32 via tiny input tensors.

Layouts (per core):
  xT   [D, B*T]  x transposed (host), bf16 - moving operand of q/k proj,
                 stationary of v proj.
  q^T,k^T kept [head_dim, tok] in SBUF; V kept [tok, head_dim];
  scores computed transposed S^T = [k-tok, q-tok] so that
  P^T = exp(S^T) feeds the y^T matmul directly (no on-chip transposes
  anywhere). Softmax denominator accumulated on DVE, reduced across
  partitions on GpSimd, broadcast back via a rank-1 matmul.
"""

import numpy as np
import ml_dtypes

B, T, D, H = 2, 2048, 2048, 16
HD = 128  # head dim
NCORES = 8
HPC = H // NCORES  # heads per core = 2
HDC = HPC * HD  # per-core projection width = 256
BT = B * T  # 4096

QT = 512  # q-tile (free dim of S^T / y^T matmuls)
KB = 128  # k-block (partition dim of S^T)

_cache = {}


def _build_nc():
    import concourse.tile as tile
    from concourse import bacc, mybir

    f32 = mybir.dt.float32
    bf16 = mybir.dt.bfloat16

    nc = bacc.Bacc("TRN2", target_bir_lowering=False, debug=False,
                   num_devices=NCORES)

    xT = nc.dram_tensor("xT", [D, BT], bf16, kind="ExternalInput").ap()
    wqT = nc.dram_tensor("wqT", [D, HDC], bf16, kind="ExternalInput").ap()
    wkT = nc.dram_tensor("wkT", [D, HDC], bf16, kind="ExternalInput").ap()
    wvT = nc.dram_tensor("wvT", [D, HDC], bf16, kind="ExternalInput").ap()
    woT = nc.dram_tensor("woT", [HDC, D], bf16, kind="ExternalInput").ap()
    # scal_qk: [128,1] filled with s_q*s_k/sqrt(HD) (folded into q^T)
    # scal_vo: [1,1] filled with 1/(s_v*s_o) (folded into softmax denom)
    scal_qk = nc.dram_tensor("scal_qk", [128, 1], f32, kind="ExternalInput").ap()
    scal_vo = nc.dram_tensor("scal_vo", [1, 1], f32, kind="ExternalInput").ap()
    out = nc.dram_tensor("out", [BT, D], bf16, kind="ExternalOutput").ap()

    with tile.TileContext(nc) as tc:
        with (
            tc.tile_pool(name="singles", bufs=1) as singles,
            tc.tile_pool(name="xstream", bufs=2) as xstream,
            tc.tile_pool(name="work", bufs=6) as work,
            tc.tile_pool(name="dwork", bufs=3) as dwork,
            tc.tile_pool(name="outsb", bufs=6) as outsb,
        ):
            # ---- persistent SBUF tensors -------------------------------
            wq_sb = singles.tile([128, D // 128, HDC], bf16, tag="wq")
            wk_sb = singles.tile([128, D // 128, HDC], bf16, tag="wk")
            wv_sb = singles.tile([128, D // 128, HDC], bf16, tag="wv")
            wo_sb = singles.tile([128, HPC, D], bf16, tag="wo")
            sqk_sb = singles.tile([128, 1], f32, tag="sqk")
            svo_sb = singles.tile([1, 1], f32, tag="svo")
            ones_col = singles.tile([128, 1], bf16, tag="ones")
            nc.vector.memset(ones_col, 1.0)
            # 0/1 causal masks for the two diagonal pair offsets
            from concourse.mybir import AluOpType as _Alu
            mask_sb = []
            for mi in range(2):
                mk = singles.tile([128, 2, QT], bf16, tag=f"mask{mi}",
                                  name=f"mask{mi}")
                nc.gpsimd.memset(mk, 1.0)
                nc.gpsimd.affine_select(
                    out=mk, in_=mk,
                    pattern=[[-KB, 2], [1, QT]],
                    channel_multiplier=-1,
                    base=-mi * 2 * KB,
                    compare_op=_Alu.is_ge,
                    fill=0.0,
                )
                mask_sb.append(mk)
            # per-batch / per-unit splits so later stages can start as soon
            # as their slice of the data is ready (Tile deps are per-tile)
            qT_sb = [singles.tile([128, HPC, T], bf16, tag=f"qT{b}", name=f"qT{b}")
                     for b in range(B)]
            kT_sb = [singles.tile([128, HPC, T], bf16, tag=f"kT{b}", name=f"kT{b}")
                     for b in range(B)]
            v_sb = [singles.tile([128, T // 128, HDC], bf16, tag=f"v{b}", name=f"v{b}")
                    for b in range(B)]
            yT_sb = [[singles.tile([128, T], bf16, tag=f"yT{b}{hh}", name=f"yT{b}{hh}")
                      for hh in range(HPC)] for b in range(B)]

            nc.gpsimd.dma_start(out=wq_sb, in_=wqT.rearrange("(c p) m -> p c m", p=128))
            nc.gpsimd.dma_start(out=wk_sb, in_=wkT.rearrange("(c p) m -> p c m", p=128))
            nc.gpsimd.dma_start(out=wv_sb, in_=wvT.rearrange("(c p) m -> p c m", p=128))
            nc.gpsimd.dma_start(out=wo_sb, in_=woT.rearrange("(c p) m -> p c m", p=128))
            nc.gpsimd.dma_start(out=sqk_sb, in_=scal_qk)
            nc.gpsimd.dma_start(out=svo_sb, in_=scal_vo)

            # ---- stage A: projections ---------------------------------
            NKK = D // 128  # 16 contraction chunks
            xTr = xT.rearrange("(c p) n -> p c n", p=128)
            with tc.tile_pool(name="psA", bufs=2, space="PSUM") as psA:
                for j in range(BT // QT):  # 8 column blocks of 512 tokens
                    b, jb = divmod(j, T // QT)
                    xt = xstream.tile([128, NKK, QT], bf16, tag="xt")
                    nc.sync.dma_start(out=xt, in_=xTr[:, :, j * QT:(j + 1) * QT])
                    # q^T, k^T : [head_dim part, tok free]
                    for (wsb, dst) in ((wq_sb, qT_sb[b]), (wk_sb, kT_sb[b])):
                        for h2 in range(HPC):
                            ps = psA.tile([128, QT], f32, tag="qk")
                            for kk in range(NKK):
                                nc.tensor.matmul(
                                    ps,
                                    lhsT=wsb[:, kk, h2 * HD:(h2 + 1) * HD],
                                    rhs=xt[:, kk, :],
                                    start=(kk == 0), stop=(kk == NKK - 1),
                                )
                            if wsb is wq_sb:
                                # fold exp scale s_q*s_k/sqrt(hd) into q^T
                                nc.vector.tensor_scalar_mul(
                                    dst[:, h2, jb * QT:(jb + 1) * QT], ps, sqk_sb)
                            else:
                                nc.any.tensor_copy(
                                    dst[:, h2, jb * QT:(jb + 1) * QT], ps)
                    # v : [tok part, head_dim free]
                    for mm in range(QT // 128):  # 4 token chunks of 128
                        ps = psA.tile([128, HDC], f32, tag="v")
                        for kk in range(NKK):
                            nc.tensor.matmul(
                                ps,
                                lhsT=xt[:, kk, mm * 128:(mm + 1) * 128],
                                rhs=wv_sb[:, kk, :],
                                start=(kk == 0), stop=(kk == NKK - 1),
                            )
                        nc.any.tensor_copy(v_sb[b][:, jb * 4 + mm, :], ps)

            # ---- stage B: causal attention ----------------------------
            from concourse.mybir import ActivationFunctionType as AF
            from concourse.mybir import AluOpType

            LOOKAHEAD = 1  # in pairs of k-blocks
            with (
                tc.tile_pool(name="psS", bufs=2, space="PSUM") as psS,
                tc.tile_pool(name="psY", bufs=3, space="PSUM") as psY,
                tc.tile_pool(name="psD", bufs=1, space="PSUM") as psD,
            ):
                for b in range(B):
                    for hh in range(HPC):
                        for iq in range(T // QT):  # 4 q-tiles of 512
                            npair = (iq + 1) * (QT // KB) // 2
                            nkb = npair * 2
                            y_ps = psY.tile([128, QT], f32, tag="y")
                            d_ps = psD.tile([1, QT], f32, tag="d")

                            def s_pair(pr):
                                # two k-blocks -> one 2-bank PSUM tile, one
                                # Exp op, one (paired) causal mask op
                                s_ps = psS.tile([128, 2, QT], f32, tag="s")
                                for j in range(2):
                                    kb = pr * 2 + j
                                    nc.tensor.matmul(
                                        s_ps[:, j, :],
                                        lhsT=kT_sb[b][:, hh, kb * KB:(kb + 1) * KB],
                                        rhs=qT_sb[b][:, hh, iq * QT:(iq + 1) * QT],
                                        start=True, stop=True,
                                    )
                                pt = work.tile([128, 2, QT], bf16, tag="pt")
                                # exp per bank half: the first half is ready
                                # for its d/y matmuls while the second runs
                                for j in range(2):
                                    nc.scalar.activation(pt[:, j, :],
                                                         s_ps[:, j, :], AF.Exp)
                                    if pr >= 2 * iq:
                                        # pair touches the diagonal: zero k>q
                                        nc.vector.tensor_mul(
                                            pt[:, j, :], pt[:, j, :],
                                            mask_sb[pr - 2 * iq][:, j, :])
                                return pt

                            pts = [s_pair(p) for p in range(min(LOOKAHEAD, npair))]
                            for pr in range(npair):
                                pt = pts[pr]
                                if pr + LOOKAHEAD < npair:
                                    pts.append(s_pair(pr + LOOKAHEAD))
                                for j in range(2):
                                    kb = pr * 2 + j
                                    # d[q] += sum_k pt[k, q] (rank-1 on PE)
                                    nc.tensor.matmul(
                                        d_ps, lhsT=ones_col, rhs=pt[:, j, :],
                                        start=(kb == 0), stop=(kb == nkb - 1),
                                    )
                                    nc.tensor.matmul(
                                        y_ps,
                                        lhsT=v_sb[b][:, kb, hh * HD:(hh + 1) * HD],
                                        rhs=pt[:, j, :],
                                        start=(kb == 0), stop=(kb == nkb - 1),
                                    )
                            # softmax denominator and normalization
                            # (DVE/GpSimd only - no PE op, so the next
                            # q-tile's matmuls proceed without stalling)
                            d_sb = dwork.tile([1, QT], f32, tag="dsb")
                            nc.vector.tensor_scalar_mul(d_sb, d_ps, svo_sb)
                            dr = dwork.tile([1, QT], f32, tag="dr")
                            nc.vector.reciprocal_approx_fast(dr, d_sb)
                            r_sb = dwork.tile([128, QT], f32, tag="rsb")
                            nc.gpsimd.partition_broadcast(r_sb, dr)
                            nc.vector.tensor_mul(
                                yT_sb[b][hh][:, iq * QT:(iq + 1) * QT],
                                y_ps, r_sb)

            # ---- stage C: out-projection (partial) --------------------
            with tc.tile_pool(name="psC", bufs=4, space="PSUM") as psC:
                for m in range(BT // 128):  # 32 token chunks
                    b, mb = divmod(m, T // 128)
                    for n in range(D // QT):  # 4 output column tiles
                        ps = psC.tile([128, QT], f32, tag="o")
                        for kk in range(HPC):
                            nc.tensor.matmul(
                                ps,
                                lhsT=yT_sb[b][kk][:, mb * 128:(mb + 1) * 128],
                                rhs=wo_sb[:, kk, n * QT:(n + 1) * QT],
                                start=(kk == 0), stop=(kk == HPC - 1),
                            )
                        o_sb = outsb.tile([128, QT], bf16, tag="osb")
                        nc.vector.tensor_copy(o_sb, ps)
                        nc.sync.dma_start(
                            out=out[m * 128:(m + 1) * 128, n * QT:(n + 1) * QT],
                            in_=o_sb)

    nc.compile()
    return nc


def kernel(x, wq, wk, wv, wo):
    import concourse.bass_utils as bass_utils

    x = np.asarray(x, dtype=np.float32)
    bf16 = ml_dtypes.bfloat16

    if "nc" not in _cache:
        _cache["nc"] = _build_nc()
    nc = _cache["nc"]

    scales = {}
    signs = {}
    for name, w in (("q", wq), ("k", wk), ("v", wv), ("o", wo)):
        w = np.asarray(w, dtype=np.float32)
        scales[name] = max(np.mean(np.abs(w)), 1e-5)
        signs[name] = np.sign(w)

    s_qk = np.float32(scales["q"] * scales["k"] / np.sqrt(HD))
    s_vo = np.float32(1.0 / (scales["v"] * scales["o"]))

    xT = np.ascontiguousarray(x.reshape(BT, D).T).astype(bf16)
    scal_qk = np.full((128, 1), s_qk, dtype=np.float32)
    scal_vo = np.full((1, 1), s_vo, dtype=np.float32)

    in_maps = []
    for c in range(NCORES):
        sl = slice(c * HDC, (c + 1) * HDC)
        in_maps.append({
            "xT": xT,
            "wqT": np.ascontiguousarray(signs["q"][sl, :].T).astype(bf16),
            "wkT": np.ascontiguousarray(signs["k"][sl, :].T).astype(bf16),
            "wvT": np.ascontiguousarray(signs["v"][sl, :].T).astype(bf16),
            "woT": np.ascontiguousarray(signs["o"][:, sl].T).astype(bf16),
            "scal_qk": scal_qk,
            "scal_vo": scal_vo,
        })

    res = bass_utils.run_bass_kernel_spmd(nc, in_maps,
                                          core_ids=list(range(NCORES)),
                                          **_cache.get("run_kwargs", {}))
    _cache["last_result"] = res

    acc = np.zeros((BT, D), dtype=np.float32)
    for r in res.results:
        acc += np.asarray(r["out"], dtype=np.float32)
    return acc.reshape(B, T, D)


# revision 17
# speedup vs baseline: 3.7282x; 1.0382x over previous
"""BitAttention (BitNet-style ternary-quantized attention) on 8 Trainium2
NeuronCores.

Sharding: tensor-parallel across heads. 16 heads / 8 cores = 2 heads per
core. Each core computes q/k/v projections for its 2 heads (output-dim
shard), causal attention for those heads, and a partial out-projection
(input-dim shard of wo). Host sums the 8 partial outputs (the all-reduce
of the hint, done at unshard time).

Weight quantization sign(w) * mean(|w|) is separable: the +-1 sign
matrices are exact in bf16 and become matmul operands; the four scalar
scales are folded into the softmax exp scale and the output scale, both
applied on-device in fp32 via tiny input tensors.

Layouts (per core):
  xT   [D, B*T]  x transposed (host), bf16 - moving operand of q/k proj,
                 stationary of v proj.
  q^T,k^T kept [head_dim, tok] in SBUF; V kept [tok, head_dim];
  scores computed transposed S^T = [k-tok, q-tok] so that
  P^T = exp(S^T) feeds the y^T matmul directly (no on-chip transposes
  anywhere). Softmax denominator accumulated on DVE, reduced across
  partitions on GpSimd, broadcast back via a rank-1 matmul.
"""

import numpy as np
import ml_dtypes

B, T, D, H = 2, 2048, 2048, 16
HD = 128  # head dim
NCORES = 8
HPC = H // NCORES  # heads per core = 2
HDC = HPC * HD  # per-core projection width = 256
BT = B * T  # 4096

QT = 512  # q-tile (free dim of S^T / y^T matmuls)
KB = 128  # k-block (partition dim of S^T)

_cache = {}


def _build_nc():
    import concourse.tile as tile
    from concourse import bacc, mybir

    f32 = mybir.dt.float32
    bf16 = mybir.dt.bfloat16
    f8 = mybir.dt.float8e4
    DR = mybir.MatmulPerfMode.DoubleRow

    nc = bacc.Bacc("TRN2", target_bir_lowering=False, debug=False,
                   num_devices=NCORES)

    xT = nc.dram_tensor("xT", [D, BT], bf16, kind="ExternalInput").ap()
    wqT = nc.dram_tensor("wqT", [D, HDC], bf16, kind="ExternalInput").ap()
    wkT = nc.dram_tensor("wkT", [D, HDC], bf16, kind="ExternalInput").ap()
    wvT = nc.dram_tensor("wvT", [D, HDC], bf16, kind="ExternalInput").ap()
    woT = nc.dram_tensor("woT", [HDC, D], bf16, kind="ExternalInput").ap()
    # scal_qk: [128,1] filled with s_q*s_k/sqrt(HD) (folded into q^T)
    # scal_vo: [1,1] filled with 1/(s_v*s_o) (folded into softmax denom)
    scal_qk = nc.dram_tensor("scal_qk", [128, 1], f32, kind="ExternalInput").ap()
    scal_vo = nc.dram_tensor("scal_vo", [1, 1], f32, kind="ExternalInput").ap()
    out = nc.dram_tensor("out", [BT, D], bf16, kind="ExternalOutput").ap()

    with tile.TileContext(nc) as tc:
        with (
            tc.tile_pool(name="singles", bufs=1) as singles,
            tc.tile_pool(name="xstream", bufs=2) as xstream,
            tc.tile_pool(name="work", bufs=6) as work,
            tc.tile_pool(name="dwork", bufs=3) as dwork,
            tc.tile_pool(name="outsb", bufs=6) as outsb,
        ):
            # ---- persistent SBUF tensors -------------------------------
            wq_sb = singles.tile([128, D // 128, HDC], bf16, tag="wq")
            wk_sb = singles.tile([128, D // 128, HDC], bf16, tag="wk")
            wv_sb = singles.tile([128, D // 128, HDC], bf16, tag="wv")
            wo_sb = singles.tile([128, HPC, D], bf16, tag="wo")
            sqk_sb = singles.tile([128, 1], f32, tag="sqk")
            svo_sb = singles.tile([1, 1], f32, tag="svo")
            ones_col = singles.tile([128, 1], bf16, tag="ones")
            nc.vector.memset(ones_col, 1.0)
            # 0/1 causal masks for the two diagonal pair offsets
            from concourse.mybir import AluOpType as _Alu
            mask_sb = []
            for mi in range(QT // KB):
                mk = singles.tile([128, QT], bf16, tag=f"mask{mi}",
                                  name=f"mask{mi}")
                nc.gpsimd.memset(mk, 1.0)
                nc.gpsimd.affine_select(
                    out=mk, in_=mk,
                    pattern=[[1, QT]],
                    channel_multiplier=-1,
                    base=-mi * KB,
                    compare_op=_Alu.is_ge,
                    fill=0.0,
                )
                mask_sb.append(mk)
            # per-batch / per-unit splits so later stages can start as soon
            # as their slice of the data is ready (Tile deps are per-tile)
            qT_sb = [singles.tile([128, HPC, T], bf16, tag=f"qT{b}", name=f"qT{b}")
                     for b in range(B)]
            kT_sb = [singles.tile([128, HPC, T], bf16, tag=f"kT{b}", name=f"kT{b}")
                     for b in range(B)]
            v_sb = [singles.tile([128, T // 128, HDC], bf16, tag=f"v{b}", name=f"v{b}")
                    for b in range(B)]
            yT_sb = [[singles.tile([128, T], bf16, tag=f"yT{b}{hh}", name=f"yT{b}{hh}")
                      for hh in range(HPC)] for b in range(B)]

            nc.gpsimd.dma_start(out=wq_sb, in_=wqT.rearrange("(c p) m -> p c m", p=128))
            nc.gpsimd.dma_start(out=wk_sb, in_=wkT.rearrange("(c p) m -> p c m", p=128))
            nc.gpsimd.dma_start(out=wv_sb, in_=wvT.rearrange("(c p) m -> p c m", p=128))
            nc.gpsimd.dma_start(out=wo_sb, in_=woT.rearrange("(c p) m -> p c m", p=128))
            nc.gpsimd.dma_start(out=sqk_sb, in_=scal_qk)
            nc.gpsimd.dma_start(out=svo_sb, in_=scal_vo)

            # ---- single whole-kernel PSUM pool --------------------------
            # tags: "s" (3 banks: stage-A accum chains + attention S tiles),
            #       "y" (2 banks: attention y accumulators),
            #       "mix" (3 banks: A chains early / softmax d + out-proj o)
            # Total 8 banks, no pool boundaries, so stages overlap freely.
            from concourse.mybir import ActivationFunctionType as AF

            NKK = D // 128  # 16 contraction chunks
            xTr = xT.rearrange("(c p) n -> p c n", p=128)
            LOOKAHEAD = 3  # k-blocks of S issued ahead of their d/y matmuls

            with tc.tile_pool(name="ps", bufs=1, space="PSUM") as psP:

                def emit_A(j):
                    b, jb = divmod(j, T // QT)
                    xt = xstream.tile([128, NKK, QT], bf16, tag="xt",
                                      name="xt")
                    nc.sync.dma_start(out=xt, in_=xTr[:, :, j * QT:(j + 1) * QT])
                    # q^T, k^T : [head_dim part, tok free]
                    for (wsb, dst) in ((wq_sb, qT_sb[b]), (wk_sb, kT_sb[b])):
                        for h2 in range(HPC):
                            ps = psP.tile([128, QT], f32, tag="mix", bufs=3,
                                          name="psA")
                            for kk in range(NKK):
                                nc.tensor.matmul(
                                    ps,
                                    lhsT=wsb[:, kk, h2 * HD:(h2 + 1) * HD],
                                    rhs=xt[:, kk, :],
                                    start=(kk == 0), stop=(kk == NKK - 1),
                                )
                            if wsb is wq_sb:
                                # fold exp scale s_q*s_k/sqrt(hd) into q^T
                                nc.vector.tensor_scalar_mul(
                                    dst[:, h2, jb * QT:(jb + 1) * QT], ps, sqk_sb)
                            else:
                                nc.any.tensor_copy(
                                    dst[:, h2, jb * QT:(jb + 1) * QT], ps)
                    # v : [tok part, head_dim free]
                    for mm in range(QT // 128):  # 4 token chunks of 128
                        ps = psP.tile([128, HDC], f32, tag="mix", bufs=3,
                                      name="psV")
                        for kk in range(NKK):
                            nc.tensor.matmul(
                                ps,
                                lhsT=xt[:, kk, mm * 128:(mm + 1) * 128],
                                rhs=wv_sb[:, kk, :],
                                start=(kk == 0), stop=(kk == NKK - 1),
                            )
                        nc.any.tensor_copy(v_sb[b][:, jb * 4 + mm, :], ps)

                def emit_B(b, hh, iq):
                    nkb = (iq + 1) * (QT // KB)
                    y_ps = psP.tile([128, QT], f32, tag="y", bufs=2, name="psYt")
                    d_ps = psP.tile([1, QT], f32, tag="mix", bufs=3, name="psDt")

                    def s_block(kb):
                        s_ps = psP.tile([128, QT], f32, tag="s", bufs=3,
                                        name="psSt")
                        nc.tensor.matmul(
                            s_ps,
                            lhsT=kT_sb[b][:, hh, kb * KB:(kb + 1) * KB],
                            rhs=qT_sb[b][:, hh, iq * QT:(iq + 1) * QT],
                            start=True, stop=True,
                        )
                        pt = work.tile([128, QT], bf16, tag="pt", name="pt")
                        nc.scalar.activation(pt, s_ps, AF.Exp)
                        if kb >= iq * (QT // KB):
                            # block touches the diagonal: zero k>q
                            nc.vector.tensor_mul(
                                pt, pt, mask_sb[kb - iq * (QT // KB)])
                        return pt

                    pts = [s_block(kb) for kb in range(min(LOOKAHEAD, nkb))]
                    for kb in range(nkb):
                        pt = pts[kb]
                        if kb + LOOKAHEAD < nkb:
                            pts.append(s_block(kb + LOOKAHEAD))
                        # d[q] += sum_k pt[k, q] (rank-1 on PE)
                        nc.tensor.matmul(
                            d_ps, lhsT=ones_col, rhs=pt,
                            start=(kb == 0), stop=(kb == nkb - 1),
                        )
                        nc.tensor.matmul(
                            y_ps,
                            lhsT=v_sb[b][:, kb, hh * HD:(hh + 1) * HD],
                            rhs=pt,
                            start=(kb == 0), stop=(kb == nkb - 1),
                        )
                    # softmax denominator + normalization (DVE/GpSimd only)
                    d_sb = dwork.tile([1, QT], f32, tag="dsb", name="dsb")
                    nc.vector.tensor_scalar_mul(d_sb, d_ps, svo_sb)
                    dr = dwork.tile([1, QT], f32, tag="dr", name="dr")
                    nc.vector.reciprocal_approx_fast(dr, d_sb)
                    r_sb = dwork.tile([128, QT], f32, tag="rsb", name="rsb")
                    nc.gpsimd.partition_broadcast(r_sb, dr)
                    nc.vector.tensor_mul(
                        yT_sb[b][hh][:, iq * QT:(iq + 1) * QT], y_ps, r_sb)

                def emit_C(b, mb):
                    for n in range(D // QT):  # 4 output column tiles
                        ps = psP.tile([128, QT], f32, tag="mix", bufs=3,
                                      name="psOt")
                        for kk in range(HPC):
                            nc.tensor.matmul(
                                ps,
                                lhsT=yT_sb[b][kk][:, mb * 128:(mb + 1) * 128],
                                rhs=wo_sb[:, kk, n * QT:(n + 1) * QT],
                                start=(kk == 0), stop=(kk == HPC - 1),
                            )
                        o_sb = outsb.tile([128, QT], bf16, tag="osb", name="osb")
                        nc.vector.tensor_copy(o_sb, ps)
                        m = b * (T // 128) + mb
                        nc.sync.dma_start(
                            out=out[m * 128:(m + 1) * 128, n * QT:(n + 1) * QT],
                            in_=o_sb)

                # emission schedule: A(b0) | A(b1) interleaved with B(b0) |
                # B(b1) interleaved with C(b0) | C(b1)
                for j in range(4):
                    emit_A(j)
                b0_tiles = [(0, hh, iq) for hh in range(HPC)
                            for iq in range(T // QT)]
                for j in range(4, 8):
                    emit_A(j)
                    for _ in range(2):
                        if b0_tiles:
                            emit_B(*b0_tiles.pop(0))
                while b0_tiles:
                    emit_B(*b0_tiles.pop(0))
                b1_tiles = [(1, hh, iq) for hh in range(HPC)
                            for iq in range(T // QT)]
                c0_tiles = list(range(T // 128))
                for bt in b1_tiles:
                    emit_B(*bt)
                    for _ in range(2):
                        if c0_tiles:
                            emit_C(0, c0_tiles.pop(0))
                while c0_tiles:
                    emit_C(0, c0_tiles.pop(0))
                for mb in range(T // 128):
                    emit_C(1, mb)

    nc.compile()
    return nc


def kernel(x, wq, wk, wv, wo):
    import concourse.bass_utils as bass_utils

    x = np.asarray(x, dtype=np.float32)
    bf16 = ml_dtypes.bfloat16

    if "nc" not in _cache:
        _cache["nc"] = _build_nc()
    nc = _cache["nc"]

    scales = {}
    signs = {}
    for name, w in (("q", wq), ("k", wk), ("v", wv), ("o", wo)):
        w = np.asarray(w, dtype=np.float32)
        scales[name] = max(np.mean(np.abs(w)), 1e-5)
        signs[name] = np.sign(w)

    s_qk = np.float32(scales["q"] * scales["k"] / np.sqrt(HD))
    s_vo = np.float32(1.0 / (scales["v"] * scales["o"]))

    xT = np.ascontiguousarray(x.reshape(BT, D).T).astype(bf16)
    scal_qk = np.full((128, 1), s_qk, dtype=np.float32)
    scal_vo = np.full((1, 1), s_vo, dtype=np.float32)

    in_maps = []
    for c in range(NCORES):
        sl = slice(c * HDC, (c + 1) * HDC)
        in_maps.append({
            "xT": xT,
            "wqT": np.ascontiguousarray(signs["q"][sl, :].T).astype(bf16),
            "wkT": np.ascontiguousarray(signs["k"][sl, :].T).astype(bf16),
            "wvT": np.ascontiguousarray(signs["v"][sl, :].T).astype(bf16),
            "woT": np.ascontiguousarray(signs["o"][:, sl].T).astype(bf16),
            "scal_qk": scal_qk,
            "scal_vo": scal_vo,
        })

    res = bass_utils.run_bass_kernel_spmd(nc, in_maps,
                                          core_ids=list(range(NCORES)),
                                          **_cache.get("run_kwargs", {}))
    _cache["last_result"] = res

    acc = np.zeros((BT, D), dtype=np.float32)
    for r in res.results:
        acc += np.asarray(r["out"], dtype=np.float32)
    return acc.reshape(B, T, D)
